# revision 5
# baseline (speedup 1.0000x reference)
"""BERT self-attention (B=2, S=2048, D=768, H=12) on 8 trn2 NeuronCores.

Sharding: core c -> batch b = c//4, head group g = c%4 (3 heads each).
Attention is fully local per core; no collectives.

Per-core program (all f32 storage; matmuls run in float32r fast mode):
  qT/kT[dout, s] = W^T.T @ x^T          (projection, transposed orientation)
  v[t, w]        = x^T.T @ Wv^T         (natural orientation, +ones column)
  scoresT[t, s]  = kT_h.T @ qT_h        (per head, per 128-t-block)
  expS           = exp(scoresT/8 + maskbias[t])   (one ACT pass, PSUM->SBUF)
  ctxT_aug[65,s] = v_aug.T @ expS accumulated over t   (row 64 = sum of exp)
  ctx[s, 65]     = PE-transpose(ctxT_aug); out = ctx[:, :64] / ctx[:, 64]
"""

import sys

import numpy as np

_TRN_REPO = "/opt/trn_rl_repo"
if _TRN_REPO not in sys.path:
    sys.path.insert(0, _TRN_REPO)

import concourse.tile as tile  # noqa: E402
from concourse import bacc, mybir  # noqa: E402
from concourse.bass_utils import run_bass_kernel_spmd  # noqa: E402

F32 = mybir.dt.float32
F32R = mybir.dt.float32r
AF = mybir.ActivationFunctionType

B, S, D = 2, 2048, 768
H_TOT, W = 12, 64
N_CORES = 8
HL = 3                # heads per core
DH = HL * W           # 192 local output dims
KC = D // 128         # 6 contraction chunks of 128
ST = 512              # s-tile (matmul moving free dim)
NS = S // ST          # 4 s-tiles
NT = S // 128         # 16 t-blocks
VPAD = 256            # v-projection free dim padded so float32r runs 1 cyc/row


def _round_f32r(a):
    """Round-to-nearest-even fp32 -> fp32r (11-bit mantissa, value kept in
    the top 20 bits of the word) so DMA'd data is already fp32r-valid."""
    u = np.ascontiguousarray(a, np.float32).view(np.uint32).copy()
    u += np.uint32(0x7FF) + ((u >> np.uint32(12)) & np.uint32(1))
    u &= np.uint32(0xFFFFF000)
    return u.view(np.float32)


def _emit(tc, aps, has_bias, has_mask):
    nc = tc.nc
    xt_d, wq_d, wk_d, wv_d, id_d, on_d, mb_d, out_d = aps

    from contextlib import ExitStack

    with ExitStack() as ctx:
        const = ctx.enter_context(tc.tile_pool(name="const", bufs=1))

        ident = const.tile([128, 128], F32, name="ident", tag="ident")
        nc.sync.dma_start(out=ident[:], in_=id_d[:, :])

        ones = const.tile([1, ST], F32R, name="ones", tag="ones")
        nc.sync.dma_start(out=ones[:], in_=on_d[0:1, :])

        mb = None
        if has_mask:
            mb = const.tile([128, NT], F32, name="mb", tag="mb")
            nc.sync.dma_start(out=mb[:], in_=mb_d[:, :])

        # x^T tiles: 6 chunks of [128 d, 2048 s], loaded per s-column-block so
        # compute can start before the whole 6.3MB lands.
        xt = []
        for c in range(KC):
            t = const.tile([128, S], F32R, name=f"xt{c}", tag=f"xt{c}")
            xt.append(t)
        for si in range(NS):
            for c in range(KC):
                nc.sync.dma_start(
                    out=xt[c][:, si * ST:(si + 1) * ST],
                    in_=xt_d[c * 128:(c + 1) * 128, si * ST:(si + 1) * ST],
                )

        def load_w(name, dram, ncols):
            chunks = []
            for c in range(KC):
                t = const.tile([128, ncols], F32R, name=f"{name}{c}", tag=f"{name}{c}")
                nc.sync.dma_start(out=t[:], in_=dram[c * 128:(c + 1) * 128, :])
                chunks.append(t)
            brow = const.tile([1, ncols], F32R, name=f"{name}b", tag=f"{name}b")
            if has_bias:
                nc.sync.dma_start(out=brow[:], in_=dram[D:D + 1, :])
            return chunks, brow

        wq, wqb = load_w("wq", wq_d, DH)
        wk, wkb = load_w("wk", wk_d, DH)
        wv, wvb = load_w("wv", wv_d, VPAD)

        # Projection outputs (persistent).
        qt_a = const.tile([128, S], F32R, name="qt_a", tag="qt_a")
        qt_b = const.tile([64, S], F32R, name="qt_b", tag="qt_b")
        kt_a = const.tile([128, S], F32R, name="kt_a", tag="kt_a")
        kt_b = const.tile([64, S], F32R, name="kt_b", tag="kt_b")
        vaug = []
        for t in range(NT):
            va = const.tile([128, HL, W + 1], F32R, name=f"vaug{t}", tag=f"vaug{t}")
            nc.sync.dma_start(
                out=va[:, :, W:W + 1],
                in_=on_d[0:128, 0:HL].rearrange("p (a b) -> p a b", b=1),
            )
            vaug.append(va)

        # ---- Phase A: projections -------------------------------------
        with tc.tile_pool(name="qkpsum", bufs=2, space="PSUM") as qkp, \
             tc.tile_pool(name="vpsum", bufs=2, space="PSUM") as vps:
            for si in range(NS):
                ssl = slice(si * ST, (si + 1) * ST)
                for chunks, brow, dst_a, dst_b in (
                    (wq, wqb, qt_a, qt_b),
                    (wk, wkb, kt_a, kt_b),
                ):
                    psA = qkp.tile([128, ST], F32, name="psA", tag="psA")
                    psB = qkp.tile([64, ST], F32, name="psB", tag="psB")
                    for c in range(KC):
                        nc.tensor.matmul(
                            psA[:], chunks[c][:, 0:128], xt[c][:, ssl],
                            start=(c == 0), stop=(c == KC - 1 and not has_bias),
                        )
                    if has_bias:
                        nc.tensor.matmul(
                            psA[:], brow[:, 0:128], ones[:],
                            start=False, stop=True,
                        )
                    for c in range(KC):
                        nc.tensor.matmul(
                            psB[:], chunks[c][:, 128:DH], xt[c][:, ssl],
                            start=(c == 0), stop=(c == KC - 1 and not has_bias),
                        )
                    if has_bias:
                        nc.tensor.matmul(
                            psB[:], brow[:, 128:DH], ones[:],
                            start=False, stop=True,
                        )
                    nc.scalar.copy(dst_a[:, ssl], psA[:])
                    nc.scalar.copy(dst_b[:, ssl], psB[:])

                # v for the 4 t-blocks inside this s range
                for t in range(si * NS, (si + 1) * NS):
                    tsl = slice(t * 128, (t + 1) * 128)
                    psV = vps.tile([128, VPAD], F32, name="psV", tag="psV")
                    for c in range(KC):
                        nc.tensor.matmul(
                            psV[:], xt[c][:, tsl], wv[c][:],
                            start=(c == 0), stop=(c == KC - 1 and not has_bias),
                        )
                    if has_bias:
                        nc.tensor.matmul(
                            psV[:], ones[:, 0:128], wvb[:],
                            start=False, stop=True,
                        )
                    nc.vector.tensor_copy(
                        vaug[t][:, :, 0:W],
                        psV[:, 0:DH].rearrange("p (h w) -> p h w", h=HL),
                    )

        # ---- Phase B: attention ---------------------------------------
        with tc.tile_pool(name="scps", bufs=3, space="PSUM") as scp, \
             tc.tile_pool(name="ctxps", bufs=2, space="PSUM") as cxp, \
             tc.tile_pool(name="trps", bufs=2, space="PSUM") as trp, \
             tc.tile_pool(name="expool", bufs=3) as exp_pool, \
             tc.tile_pool(name="ctxsb", bufs=2) as csb, \
             tc.tile_pool(name="outsb", bufs=3) as osb:

            pending = []  # deferred epilogue from the previous (h, si)

            def flush_epilogue():
                while pending:
                    pending.pop(0)()

            def epilogue(h, si, ctx_ps):
                def run():
                    ctx_sb = csb.tile([65, ST], F32, name="ctx_sb", tag="ctx_sb")
                    nc.vector.tensor_copy(ctx_sb[:], ctx_ps[:])
                    for j in range(NS):
                        jsl = slice(j * 128, (j + 1) * 128)
                        tr_ps = trp.tile([128, 65], F32, name="tr_ps", tag="tr_ps")
                        nc.tensor.transpose(
                            tr_ps[:], ctx_sb[:, jsl], ident[0:65, 0:65]
                        )
                        rec = osb.tile([128, 1], F32, name="rec", tag="rec")
                        nc.vector.reciprocal(rec[:], tr_ps[:, W:W + 1])
                        outt = osb.tile([128, W], F32, name="outt", tag="outt")
                        nc.vector.tensor_scalar_mul(outt[:], tr_ps[:, 0:W], rec[:])
                        nc.sync.dma_start(
                            out=out_d[si * ST + j * 128:si * ST + (j + 1) * 128,
                                      h * W:(h + 1) * W],
                            in_=outt[:],
                        )
                return run

            for h in range(HL):
                if h < 2:
                    ktile, qtile, base = kt_a, qt_a, h * 64
                else:
                    ktile, qtile, base = kt_b, qt_b, 0
                for si in range(NS):
                    ssl = slice(si * ST, (si + 1) * ST)
                    ctx_ps = cxp.tile([65, ST], F32, name="ctx_ps", tag="ctx_ps")
                    prev_ex = None
                    prev_t = -1
                    n_acc = 0
                    for t in range(NT):
                        sc_ps = scp.tile([128, ST], F32, name="sc_ps", tag="sc_ps")
                        nc.tensor.matmul(
                            sc_ps[:],
                            ktile[base:base + 64, t * 128:(t + 1) * 128],
                            qtile[base:base + 64, ssl],
                            start=True, stop=True,
                        )
                        ex = exp_pool.tile([128, ST], F32R, name="ex", tag="ex")
                        nc.scalar.activation(
                            ex[:], sc_ps[:], AF.Exp,
                            bias=(mb[:, t:t + 1] if has_mask else 0.0),
                            scale=0.125,
                        )
                        if prev_ex is not None:
                            nc.tensor.matmul(
                                ctx_ps[:], vaug[prev_t][:, h, :], prev_ex[:],
                                start=(n_acc == 0), stop=False,
                            )
                            n_acc += 1
                        prev_ex, prev_t = ex, t
                        if t == 2:
                            flush_epilogue()
                    nc.tensor.matmul(
                        ctx_ps[:], vaug[prev_t][:, h, :], prev_ex[:],
                        start=(n_acc == 0), stop=True,
                    )
                    pending.append(epilogue(h, si, ctx_ps))
            flush_epilogue()


def _build(has_bias, has_mask):
    nc = bacc.Bacc(
        "TRN2", target_bir_lowering=False, debug=False, num_devices=N_CORES
    )
    xt_d = nc.dram_tensor("xt", [D, S], F32R, kind="ExternalInput").ap()
    wq_d = nc.dram_tensor("wq", [D + 1, DH], F32R, kind="ExternalInput").ap()
    wk_d = nc.dram_tensor("wk", [D + 1, DH], F32R, kind="ExternalInput").ap()
    wv_d = nc.dram_tensor("wv", [D + 1, VPAD], F32R, kind="ExternalInput").ap()
    id_d = nc.dram_tensor("ident", [128, 128], F32, kind="ExternalInput").ap()
    on_d = nc.dram_tensor("onesd", [128, ST], F32R, kind="ExternalInput").ap()
    mb_d = (
        nc.dram_tensor("mb", [128, NT], F32, kind="ExternalInput").ap()
        if has_mask else None
    )
    out_d = nc.dram_tensor("out", [S, DH], F32, kind="ExternalOutput").ap()

    with tile.TileContext(nc) as tc:
        _emit(tc, (xt_d, wq_d, wk_d, wv_d, id_d, on_d, mb_d, out_d),
              has_bias, has_mask)
    nc.compile()
    return nc


_NC_CACHE = {}


def _get_nc(has_bias, has_mask):
    key = (has_bias, has_mask)
    if key not in _NC_CACHE:
        _NC_CACHE[key] = _build(has_bias, has_mask)
    return _NC_CACHE[key]


def _in_maps(x, Wq, bq, Wk, bk, Wv, bv, mask, has_bias, has_mask):
    ident = np.eye(128, dtype=np.float32)
    xt_by_b = [np.ascontiguousarray(x[b].T) for b in range(B)]
    mb_by_b = [
        np.ascontiguousarray(
            ((np.asarray(mask[b]) == 0).astype(np.float32) * np.float32(-1e30))
            .reshape(NT, 128).T
        )
        for b in range(B)
    ]
    maps = []
    for c in range(N_CORES):
        b, g = divmod(c, N_CORES // B)
        lo = g * DH
        wq_a = np.empty((D + 1, DH), np.float32)
        wq_a[:D] = Wq[lo:lo + DH, :].T
        wq_a[D] = bq[lo:lo + DH]
        wk_a = np.empty((D + 1, DH), np.float32)
        wk_a[:D] = Wk[lo:lo + DH, :].T
        wk_a[D] = bk[lo:lo + DH]
        wv_a = np.zeros((D + 1, VPAD), np.float32)
        wv_a[:D, :DH] = Wv[lo:lo + DH, :].T
        wv_a[D, :DH] = bv[lo:lo + DH]
        m = {
            "xt": _round_f32r(xt_by_b[b]), "wq": _round_f32r(wq_a),
            "wk": _round_f32r(wk_a), "wv": _round_f32r(wv_a), "ident": ident,
            "onesd": np.ones((128, ST), np.float32),
        }
        if has_mask:
            m["mb"] = mb_by_b[b]
        maps.append(m)
    return maps


def _install_ntff_hook():
    """Best-effort: make trace=True work under axon by supplying the
    antenv.axon_hooks shim the boot code degrades without."""
    import types

    try:
        from antenv.axon_hooks import get_axon_ntff_profile_hook  # noqa: F401
        return True
    except ImportError:
        pass
    try:
        import antenv
        from trn_agent_boot.trn_boot import _ntff_profile_via_ctypes

        hook = _ntff_profile_via_ctypes("/opt/axon/libaxon_pjrt.so")
        if hook is None:
            return False
        mod = types.ModuleType("antenv.axon_hooks")
        state = {"hook": hook}
        mod.get_axon_ntff_profile_hook = lambda: state["hook"]
        mod.set_axon_ntff_profile_hook = lambda h: state.update(hook=h)
        sys.modules["antenv.axon_hooks"] = mod
        antenv.axon_hooks = mod
        return True
    except Exception:
        return False


def _run(x, Wq, bq, Wk, bk, Wv, bv, mask, trace=False):
    if trace:
        trace = _install_ntff_hook()
    x = np.ascontiguousarray(np.asarray(x, np.float32))
    Wq = np.asarray(Wq, np.float32)
    Wk = np.asarray(Wk, np.float32)
    Wv = np.asarray(Wv, np.float32)
    bq = np.asarray(bq, np.float32)
    bk = np.asarray(bk, np.float32)
    bv = np.asarray(bv, np.float32)
    has_bias = bool(np.any(bq) or np.any(bk) or np.any(bv))
    has_mask = bool((np.asarray(mask) == 0).any())
    nc = _get_nc(has_bias, has_mask)
    maps = _in_maps(x, Wq, bq, Wk, bk, Wv, bv, mask, has_bias, has_mask)
    res = run_bass_kernel_spmd(nc, maps, list(range(N_CORES)), trace=trace)
    out = np.empty((B, S, D), np.float32)
    for c in range(N_CORES):
        b, g = divmod(c, N_CORES // B)
        out[b, :, g * DH:(g + 1) * DH] = res.results[c]["out"]
    return out, res


def kernel(x, Wq, bq, Wk, bk, Wv, bv, mask):
    out, _ = _run(x, Wq, bq, Wk, bk, Wv, bv, mask)
    return out


# revision 7
# speedup vs baseline: 1.5760x; 1.5760x over previous
"""BERT self-attention (B=2, S=2048, D=768, H=12) on 8 trn2 NeuronCores.

Sharding: core c -> batch b = c//4, head group g = c%4 (3 heads each).
Attention is fully local per core; no collectives.

Per-core program (all f32 storage; matmuls run in float32r fast mode):
  qT/kT[dout, s] = W^T.T @ x^T          (projection, transposed orientation)
  v[t, w]        = x^T.T @ Wv^T         (natural orientation, +ones column)
  scoresT[t, s]  = kT_h.T @ qT_h        (per head, per 128-t-block)
  expS           = exp(scoresT/8 + maskbias[t])   (one ACT pass, PSUM->SBUF)
  ctxT_aug[65,s] = v_aug.T @ expS accumulated over t   (row 64 = sum of exp)
  ctx[s, 65]     = PE-transpose(ctxT_aug); out = ctx[:, :64] / ctx[:, 64]
"""

import sys

import numpy as np

_TRN_REPO = "/opt/trn_rl_repo"
if _TRN_REPO not in sys.path:
    sys.path.insert(0, _TRN_REPO)

import concourse.tile as tile  # noqa: E402
from concourse import bacc, mybir  # noqa: E402
from concourse.bass_utils import run_bass_kernel_spmd  # noqa: E402

F32 = mybir.dt.float32
F32R = mybir.dt.float32r
AF = mybir.ActivationFunctionType

B, S, D = 2, 2048, 768
H_TOT, W = 12, 64
N_CORES = 8
HL = 3                # heads per core
DH = HL * W           # 192 local output dims
KC = D // 128         # 6 contraction chunks of 128
ST = 512              # s-tile (matmul moving free dim)
NS = S // ST          # 4 s-tiles
NT = S // 128         # 16 t-blocks
VPAD = 256            # v-projection free dim padded so float32r runs 1 cyc/row


def _round_f32r(a):
    """Round-to-nearest-even fp32 -> fp32r (11-bit mantissa, value kept in
    the top 20 bits of the word) so DMA'd data is already fp32r-valid."""
    u = np.ascontiguousarray(a, np.float32).view(np.uint32).copy()
    u += np.uint32(0x7FF) + ((u >> np.uint32(12)) & np.uint32(1))
    u &= np.uint32(0xFFFFF000)
    return u.view(np.float32)


def _emit(tc, aps, has_bias, has_mask):
    nc = tc.nc
    xt_d, wq_d, wk_d, wv_d, id_d, on_d, zr_d, mb_d, out_d = aps

    from contextlib import ExitStack

    with ExitStack() as ctx:
        const = ctx.enter_context(tc.tile_pool(name="const", bufs=1))

        ident = const.tile([128, 128], F32, name="ident", tag="ident")
        nc.sync.dma_start(out=ident[:], in_=id_d[:, :])

        ones = const.tile([1, ST], F32R, name="ones", tag="ones")
        nc.sync.dma_start(out=ones[:], in_=on_d[0:1, :])

        mb = None
        if has_mask:
            mb = const.tile([128, NT], F32, name="mb", tag="mb")
            nc.sync.dma_start(out=mb[:], in_=mb_d[:, :])

        # x^T tiles: 6 chunks of [128 d, 2048 s], loaded per s-column-block so
        # compute can start before the whole 6.3MB lands.
        xt = []
        for c in range(KC):
            t = const.tile([128, S], F32R, name=f"xt{c}", tag=f"xt{c}")
            xt.append(t)
        dmae = [nc.sync, nc.scalar, nc.gpsimd]
        for si in range(NS):
            for c in range(KC):
                dmae[c % 3].dma_start(
                    out=xt[c][:, si * ST:(si + 1) * ST],
                    in_=xt_d[c * 128:(c + 1) * 128, si * ST:(si + 1) * ST],
                )

        def load_w(name, dram, ncols):
            chunks = []
            for c in range(KC):
                t = const.tile([128, ncols], F32R, name=f"{name}{c}", tag=f"{name}{c}")
                dmae[(c + 1) % 3].dma_start(out=t[:], in_=dram[c * 128:(c + 1) * 128, :])
                chunks.append(t)
            brow = const.tile([1, ncols], F32R, name=f"{name}b", tag=f"{name}b")
            if has_bias:
                nc.sync.dma_start(out=brow[:], in_=dram[D:D + 1, :])
            return chunks, brow

        wq, wqb = load_w("wq", wq_d, DH)
        wk, wkb = load_w("wk", wk_d, DH)
        wv, wvb = load_w("wv", wv_d, VPAD)

        # Projection outputs (persistent). q tiles are zero-padded on the
        # complementary 64 partitions so every scores matmul runs K=128
        # (no PE row-mode switches mid-loop).
        qt_h = []
        for h in range(HL):
            t = const.tile([128, S], F32R, name=f"qt_h{h}", tag=f"qt_h{h}")
            qt_h.append(t)
        kt_a = const.tile([128, S], F32R, name="kt_a", tag="kt_a")
        kt_b = const.tile([128, S], F32R, name="kt_b", tag="kt_b")
        nc.sync.dma_start(out=qt_h[0][64:128, :], in_=zr_d[:, :])
        nc.scalar.dma_start(out=qt_h[1][0:64, :], in_=zr_d[:, :])
        nc.gpsimd.dma_start(out=qt_h[2][64:128, :], in_=zr_d[:, :])
        nc.gpsimd.dma_start(out=kt_b[64:128, :], in_=zr_d[:, :])
        vaug = []
        for t in range(NT):
            va = const.tile([128, HL, W + 1], F32R, name=f"vaug{t}", tag=f"vaug{t}")
            nc.sync.dma_start(
                out=va[:, :, W:W + 1],
                in_=on_d[0:128, 0:HL].rearrange("p (a b) -> p a b", b=1),
            )
            vaug.append(va)

        # ---- Phase A: projections -------------------------------------
        with tc.tile_pool(name="qkpsum", bufs=2, space="PSUM") as qkp, \
             tc.tile_pool(name="vpsum", bufs=2, space="PSUM") as vps:
            for si in range(NS):
                ssl = slice(si * ST, (si + 1) * ST)
                for chunks, brow, which in ((wq, wqb, "q"), (wk, wkb, "k")):
                    psA = qkp.tile([128, ST], F32, name="psA", tag="psA")
                    psB = qkp.tile([64, ST], F32, name="psB", tag="psB")
                    for c in range(KC):
                        nc.tensor.matmul(
                            psA[:], chunks[c][:, 0:128], xt[c][:, ssl],
                            start=(c == 0), stop=(c == KC - 1 and not has_bias),
                        )
                    if has_bias:
                        nc.tensor.matmul(
                            psA[:], brow[:, 0:128], ones[:],
                            start=False, stop=True,
                        )
                    for c in range(KC):
                        nc.tensor.matmul(
                            psB[:], chunks[c][:, 128:DH], xt[c][:, ssl],
                            start=(c == 0), stop=(c == KC - 1 and not has_bias),
                        )
                    if has_bias:
                        nc.tensor.matmul(
                            psB[:], brow[:, 128:DH], ones[:],
                            start=False, stop=True,
                        )
                    if which == "q":
                        nc.scalar.copy(qt_h[0][0:64, ssl], psA[0:64, :])
                        nc.scalar.copy(qt_h[1][64:128, ssl], psA[64:128, :])
                        nc.scalar.copy(qt_h[2][0:64, ssl], psB[:, :])
                    else:
                        nc.scalar.copy(kt_a[:, ssl], psA[:])
                        nc.scalar.copy(kt_b[0:64, ssl], psB[:, :])

                # v for the 4 t-blocks inside this s range
                for t in range(si * NS, (si + 1) * NS):
                    tsl = slice(t * 128, (t + 1) * 128)
                    psV = vps.tile([128, VPAD], F32, name="psV", tag="psV")
                    for c in range(KC):
                        nc.tensor.matmul(
                            psV[:], xt[c][:, tsl], wv[c][:],
                            start=(c == 0), stop=(c == KC - 1 and not has_bias),
                        )
                    if has_bias:
                        nc.tensor.matmul(
                            psV[:], ones[:, 0:128], wvb[:],
                            start=False, stop=True,
                        )
                    nc.vector.tensor_copy(
                        vaug[t][:, :, 0:W],
                        psV[:, 0:DH].rearrange("p (h w) -> p h w", h=HL),
                    )

        # ---- Phase B: attention ---------------------------------------
        with tc.tile_pool(name="scps", bufs=3, space="PSUM") as scp, \
             tc.tile_pool(name="ctxps", bufs=2, space="PSUM") as cxp, \
             tc.tile_pool(name="trps", bufs=2, space="PSUM") as trp, \
             tc.tile_pool(name="expool", bufs=3) as exp_pool, \
             tc.tile_pool(name="ctxsb", bufs=2) as csb, \
             tc.tile_pool(name="outsb", bufs=3) as osb:

            pending = []  # deferred epilogue from the previous (h, si)

            def flush_epilogue():
                while pending:
                    pending.pop(0)()

            def epilogue(h, si, ctx_ps):
                def run():
                    ctx_sb = csb.tile([65, ST], F32, name="ctx_sb", tag="ctx_sb")
                    nc.vector.tensor_copy(ctx_sb[:], ctx_ps[:])
                    for j in range(NS):
                        jsl = slice(j * 128, (j + 1) * 128)
                        tr_ps = trp.tile([128, 65], F32, name="tr_ps", tag="tr_ps")
                        nc.tensor.transpose(
                            tr_ps[:], ctx_sb[:, jsl], ident[0:65, 0:65]
                        )
                        rec = osb.tile([128, 1], F32, name="rec", tag="rec")
                        nc.vector.reciprocal(rec[:], tr_ps[:, W:W + 1])
                        outt = osb.tile([128, W], F32, name="outt", tag="outt")
                        nc.vector.tensor_scalar_mul(outt[:], tr_ps[:, 0:W], rec[:])
                        nc.sync.dma_start(
                            out=out_d[si * ST + j * 128:si * ST + (j + 1) * 128,
                                      h * W:(h + 1) * W],
                            in_=outt[:],
                        )
                return run

            for h in range(HL):
                ktile = kt_a if h < 2 else kt_b
                qtile = qt_h[h]
                for si in range(NS):
                    ssl = slice(si * ST, (si + 1) * ST)
                    ctx_ps = cxp.tile([65, ST], F32, name="ctx_ps", tag="ctx_ps")
                    prev_ex = None
                    prev_t = -1
                    n_acc = 0
                    for t in range(NT):
                        sc_ps = scp.tile([128, ST], F32, name="sc_ps", tag="sc_ps")
                        nc.tensor.matmul(
                            sc_ps[:],
                            ktile[:, t * 128:(t + 1) * 128],
                            qtile[:, ssl],
                            start=True, stop=True,
                        )
                        ex = exp_pool.tile([128, ST], F32R, name="ex", tag="ex")
                        nc.scalar.activation(
                            ex[:], sc_ps[:], AF.Exp,
                            bias=(mb[:, t:t + 1] if has_mask else 0.0),
                            scale=0.125,
                        )
                        if prev_ex is not None:
                            nc.tensor.matmul(
                                ctx_ps[:], vaug[prev_t][:, h, :], prev_ex[:],
                                start=(n_acc == 0), stop=False,
                            )
                            n_acc += 1
                        prev_ex, prev_t = ex, t
                        if t == 2:
                            flush_epilogue()
                    nc.tensor.matmul(
                        ctx_ps[:], vaug[prev_t][:, h, :], prev_ex[:],
                        start=(n_acc == 0), stop=True,
                    )
                    pending.append(epilogue(h, si, ctx_ps))
            flush_epilogue()


def _build(has_bias, has_mask):
    nc = bacc.Bacc(
        "TRN2", target_bir_lowering=False, debug=False, num_devices=N_CORES
    )
    xt_d = nc.dram_tensor("xt", [D, S], F32R, kind="ExternalInput").ap()
    wq_d = nc.dram_tensor("wq", [D + 1, DH], F32R, kind="ExternalInput").ap()
    wk_d = nc.dram_tensor("wk", [D + 1, DH], F32R, kind="ExternalInput").ap()
    wv_d = nc.dram_tensor("wv", [D + 1, VPAD], F32R, kind="ExternalInput").ap()
    id_d = nc.dram_tensor("ident", [128, 128], F32, kind="ExternalInput").ap()
    on_d = nc.dram_tensor("onesd", [128, ST], F32R, kind="ExternalInput").ap()
    zr_d = nc.dram_tensor("zerod", [64, S], F32R, kind="ExternalInput").ap()
    mb_d = (
        nc.dram_tensor("mb", [128, NT], F32, kind="ExternalInput").ap()
        if has_mask else None
    )
    out_d = nc.dram_tensor("out", [S, DH], F32, kind="ExternalOutput").ap()

    with tile.TileContext(nc) as tc:
        _emit(tc, (xt_d, wq_d, wk_d, wv_d, id_d, on_d, zr_d, mb_d, out_d),
              has_bias, has_mask)
    nc.compile()
    return nc


_NC_CACHE = {}


def _get_nc(has_bias, has_mask):
    key = (has_bias, has_mask)
    if key not in _NC_CACHE:
        _NC_CACHE[key] = _build(has_bias, has_mask)
    return _NC_CACHE[key]


def _in_maps(x, Wq, bq, Wk, bk, Wv, bv, mask, has_bias, has_mask):
    ident = np.eye(128, dtype=np.float32)
    xt_by_b = [np.ascontiguousarray(x[b].T) for b in range(B)]
    mb_by_b = [
        np.ascontiguousarray(
            ((np.asarray(mask[b]) == 0).astype(np.float32) * np.float32(-1e30))
            .reshape(NT, 128).T
        )
        for b in range(B)
    ]
    maps = []
    for c in range(N_CORES):
        b, g = divmod(c, N_CORES // B)
        lo = g * DH
        wq_a = np.empty((D + 1, DH), np.float32)
        wq_a[:D] = Wq[lo:lo + DH, :].T
        wq_a[D] = bq[lo:lo + DH]
        wk_a = np.empty((D + 1, DH), np.float32)
        wk_a[:D] = Wk[lo:lo + DH, :].T
        wk_a[D] = bk[lo:lo + DH]
        wv_a = np.zeros((D + 1, VPAD), np.float32)
        wv_a[:D, :DH] = Wv[lo:lo + DH, :].T
        wv_a[D, :DH] = bv[lo:lo + DH]
        m = {
            "xt": _round_f32r(xt_by_b[b]), "wq": _round_f32r(wq_a),
            "wk": _round_f32r(wk_a), "wv": _round_f32r(wv_a), "ident": ident,
            "onesd": np.ones((128, ST), np.float32),
            "zerod": np.zeros((64, S), np.float32),
        }
        if has_mask:
            m["mb"] = mb_by_b[b]
        maps.append(m)
    return maps


def _install_ntff_hook():
    """Best-effort: make trace=True work under axon by supplying the
    antenv.axon_hooks shim the boot code degrades without."""
    import types

    try:
        from antenv.axon_hooks import get_axon_ntff_profile_hook  # noqa: F401
        return True
    except ImportError:
        pass
    try:
        import antenv
        from trn_agent_boot.trn_boot import _ntff_profile_via_ctypes

        hook = _ntff_profile_via_ctypes("/opt/axon/libaxon_pjrt.so")
        if hook is None:
            return False
        mod = types.ModuleType("antenv.axon_hooks")
        state = {"hook": hook}
        mod.get_axon_ntff_profile_hook = lambda: state["hook"]
        mod.set_axon_ntff_profile_hook = lambda h: state.update(hook=h)
        sys.modules["antenv.axon_hooks"] = mod
        antenv.axon_hooks = mod
        return True
    except Exception:
        return False


def _run(x, Wq, bq, Wk, bk, Wv, bv, mask, trace=False):
    if trace:
        trace = _install_ntff_hook()
    x = np.ascontiguousarray(np.asarray(x, np.float32))
    Wq = np.asarray(Wq, np.float32)
    Wk = np.asarray(Wk, np.float32)
    Wv = np.asarray(Wv, np.float32)
    bq = np.asarray(bq, np.float32)
    bk = np.asarray(bk, np.float32)
    bv = np.asarray(bv, np.float32)
    has_bias = bool(np.any(bq) or np.any(bk) or np.any(bv))
    has_mask = bool((np.asarray(mask) == 0).any())
    nc = _get_nc(has_bias, has_mask)
    maps = _in_maps(x, Wq, bq, Wk, bk, Wv, bv, mask, has_bias, has_mask)
    res = run_bass_kernel_spmd(nc, maps, list(range(N_CORES)), trace=trace)
    out = np.empty((B, S, D), np.float32)
    for c in range(N_CORES):
        b, g = divmod(c, N_CORES // B)
        out[b, :, g * DH:(g + 1) * DH] = res.results[c]["out"]
    return out, res


def kernel(x, Wq, bq, Wk, bk, Wv, bv, mask):
    out, _ = _run(x, Wq, bq, Wk, bk, Wv, bv, mask)
    return out


# revision 8
# speedup vs baseline: 1.6441x; 1.0432x over previous
"""BERT self-attention (B=2, S=2048, D=768, H=12) on 8 trn2 NeuronCores.

Sharding: core c -> batch b = c//4, head group g = c%4 (3 heads each).
Attention is fully local per core; no collectives.

Per-core program (all f32 storage; matmuls run in float32r fast mode):
  qT/kT[dout, s] = W^T.T @ x^T          (projection, transposed orientation)
  v[t, w]        = x^T.T @ Wv^T         (natural orientation, +ones column)
  scoresT[t, s]  = kT_h.T @ qT_h        (per head, per 128-t-block)
  expS           = exp(scoresT/8 + maskbias[t])   (one ACT pass, PSUM->SBUF)
  ctxT_aug[65,s] = v_aug.T @ expS accumulated over t   (row 64 = sum of exp)
  ctx[s, 65]     = PE-transpose(ctxT_aug); out = ctx[:, :64] / ctx[:, 64]
"""

import sys

import numpy as np

_TRN_REPO = "/opt/trn_rl_repo"
if _TRN_REPO not in sys.path:
    sys.path.insert(0, _TRN_REPO)

import concourse.tile as tile  # noqa: E402
from concourse import bacc, mybir  # noqa: E402
from concourse.bass_utils import run_bass_kernel_spmd  # noqa: E402

F32 = mybir.dt.float32
F32R = mybir.dt.float32r
AF = mybir.ActivationFunctionType

B, S, D = 2, 2048, 768
H_TOT, W = 12, 64
N_CORES = 8
HL = 3                # heads per core
DH = HL * W           # 192 local output dims
KC = D // 128         # 6 contraction chunks of 128
ST = 512              # s-tile (matmul moving free dim)
NS = S // ST          # 4 s-tiles
NT = S // 128         # 16 t-blocks
VPAD = 256            # v-projection free dim padded so float32r runs 1 cyc/row


def _round_f32r(a):
    """Round-to-nearest-even fp32 -> fp32r (11-bit mantissa, value kept in
    the top 20 bits of the word) so DMA'd data is already fp32r-valid."""
    u = np.ascontiguousarray(a, np.float32).view(np.uint32).copy()
    u += np.uint32(0x7FF) + ((u >> np.uint32(12)) & np.uint32(1))
    u &= np.uint32(0xFFFFF000)
    return u.view(np.float32)


def _emit(tc, aps, has_bias, has_mask):
    nc = tc.nc
    xt_d, wq_d, wk_d, wv_d, id_d, on_d, zr_d, mb_d, out_d = aps

    from contextlib import ExitStack

    with ExitStack() as ctx:
        const = ctx.enter_context(tc.tile_pool(name="const", bufs=1))

        ident = const.tile([128, 128], F32, name="ident", tag="ident")
        nc.sync.dma_start(out=ident[:], in_=id_d[:, :])

        ones = const.tile([1, ST], F32R, name="ones", tag="ones")
        nc.sync.dma_start(out=ones[:], in_=on_d[0:1, :])

        mb = None
        if has_mask:
            mb = const.tile([128, NT], F32, name="mb", tag="mb")
            nc.sync.dma_start(out=mb[:], in_=mb_d[:, :])

        # x^T tiles: 6 chunks of [128 d, 2048 s], loaded per s-column-block so
        # compute can start before the whole 6.3MB lands.
        xt = []
        for c in range(KC):
            t = const.tile([128, S], F32R, name=f"xt{c}", tag=f"xt{c}")
            xt.append(t)
        dmae = [nc.sync, nc.scalar, nc.gpsimd]

        def load_xt_cols(si):
            for c in range(KC):
                dmae[c % 3].dma_start(
                    out=xt[c][:, si * ST:(si + 1) * ST],
                    in_=xt_d[c * 128:(c + 1) * 128, si * ST:(si + 1) * ST],
                )

        load_xt_cols(0)

        def load_w(name, dram, ncols):
            chunks = []
            for c in range(KC):
                t = const.tile([128, ncols], F32R, name=f"{name}{c}", tag=f"{name}{c}")
                dmae[(c + 1) % 3].dma_start(out=t[:], in_=dram[c * 128:(c + 1) * 128, :])
                chunks.append(t)
            brow = const.tile([1, ncols], F32R, name=f"{name}b", tag=f"{name}b")
            if has_bias:
                nc.sync.dma_start(out=brow[:], in_=dram[D:D + 1, :])
            return chunks, brow

        wq, wqb = load_w("wq", wq_d, DH)
        wk, wkb = load_w("wk", wk_d, DH)
        wv, wvb = load_w("wv", wv_d, VPAD)
        for si in range(1, NS):
            load_xt_cols(si)

        # Projection outputs (persistent). q tiles are zero-padded on the
        # complementary 64 partitions so every scores matmul runs K=128
        # (no PE row-mode switches mid-loop).
        qt_h = []
        for h in range(HL):
            t = const.tile([128, S], F32R, name=f"qt_h{h}", tag=f"qt_h{h}")
            qt_h.append(t)
        kt_a = const.tile([128, S], F32R, name="kt_a", tag="kt_a")
        kt_b = const.tile([128, S], F32R, name="kt_b", tag="kt_b")
        nc.sync.dma_start(out=qt_h[0][64:128, :], in_=zr_d[:, :])
        nc.scalar.dma_start(out=qt_h[1][0:64, :], in_=zr_d[:, :])
        nc.gpsimd.dma_start(out=qt_h[2][64:128, :], in_=zr_d[:, :])
        nc.gpsimd.dma_start(out=kt_b[64:128, :], in_=zr_d[:, :])
        vaug = []
        for t in range(NT):
            va = const.tile([128, HL, W + 1], F32R, name=f"vaug{t}", tag=f"vaug{t}")
            nc.sync.dma_start(
                out=va[:, :, W:W + 1],
                in_=on_d[0:128, 0:HL].rearrange("p (a b) -> p a b", b=1),
            )
            vaug.append(va)

        # ---- Phase A: projections -------------------------------------
        with tc.tile_pool(name="qkpsum", bufs=2, space="PSUM") as qkp, \
             tc.tile_pool(name="vpsum", bufs=2, space="PSUM") as vps:
            for si in range(NS):
                ssl = slice(si * ST, (si + 1) * ST)
                for chunks, brow, which in ((wq, wqb, "q"), (wk, wkb, "k")):
                    psA = qkp.tile([128, ST], F32, name="psA", tag="psA")
                    psB = qkp.tile([64, ST], F32, name="psB", tag="psB")
                    for c in range(KC):
                        nc.tensor.matmul(
                            psA[:], chunks[c][:, 0:128], xt[c][:, ssl],
                            start=(c == 0), stop=(c == KC - 1 and not has_bias),
                        )
                    if has_bias:
                        nc.tensor.matmul(
                            psA[:], brow[:, 0:128], ones[:],
                            start=False, stop=True,
                        )
                    for c in range(KC):
                        nc.tensor.matmul(
                            psB[:], chunks[c][:, 128:DH], xt[c][:, ssl],
                            start=(c == 0), stop=(c == KC - 1 and not has_bias),
                        )
                    if has_bias:
                        nc.tensor.matmul(
                            psB[:], brow[:, 128:DH], ones[:],
                            start=False, stop=True,
                        )
                    if which == "q":
                        nc.scalar.copy(qt_h[0][0:64, ssl], psA[0:64, :])
                        nc.scalar.copy(qt_h[1][64:128, ssl], psA[64:128, :])
                        nc.scalar.copy(qt_h[2][0:64, ssl], psB[:, :])
                    else:
                        nc.scalar.copy(kt_a[:, ssl], psA[:])
                        nc.scalar.copy(kt_b[0:64, ssl], psB[:, :])

                # v for the 4 t-blocks inside this s range
                for t in range(si * NS, (si + 1) * NS):
                    tsl = slice(t * 128, (t + 1) * 128)
                    psV = vps.tile([128, VPAD], F32, name="psV", tag="psV")
                    for c in range(KC):
                        nc.tensor.matmul(
                            psV[:], xt[c][:, tsl], wv[c][:],
                            start=(c == 0), stop=(c == KC - 1 and not has_bias),
                        )
                    if has_bias:
                        nc.tensor.matmul(
                            psV[:], ones[:, 0:128], wvb[:],
                            start=False, stop=True,
                        )
                    nc.vector.tensor_copy(
                        vaug[t][:, :, 0:W],
                        psV[:, 0:DH].rearrange("p (h w) -> p h w", h=HL),
                    )

        # ---- Phase B: attention ---------------------------------------
        exw = 1 if has_mask else 2   # t-blocks per exp tile
        with tc.tile_pool(name="scps", bufs=(3 if has_mask else 2), space="PSUM") as scp, \
             tc.tile_pool(name="ctxps", bufs=2, space="PSUM") as cxp, \
             tc.tile_pool(name="trps", bufs=2, space="PSUM") as trp, \
             tc.tile_pool(name="expool", bufs=3) as exp_pool, \
             tc.tile_pool(name="ctxsb", bufs=2) as csb, \
             tc.tile_pool(name="outsb", bufs=3) as osb:

            pending = []  # deferred epilogue from the previous (h, si)

            def flush_epilogue():
                while pending:
                    pending.pop(0)()

            def epilogue(h, si, ctx_ps):
                def run():
                    ctx_sb = csb.tile([65, ST], F32, name="ctx_sb", tag="ctx_sb")
                    nc.vector.tensor_copy(ctx_sb[:], ctx_ps[:])
                    for j in range(NS):
                        jsl = slice(j * 128, (j + 1) * 128)
                        tr_ps = trp.tile([128, 65], F32, name="tr_ps", tag="tr_ps")
                        nc.tensor.transpose(
                            tr_ps[:], ctx_sb[:, jsl], ident[0:65, 0:65]
                        )
                        rec = osb.tile([128, 1], F32, name="rec", tag="rec")
                        nc.vector.reciprocal(rec[:], tr_ps[:, W:W + 1])
                        outt = osb.tile([128, W], F32, name="outt", tag="outt")
                        nc.vector.tensor_scalar_mul(outt[:], tr_ps[:, 0:W], rec[:])
                        nc.sync.dma_start(
                            out=out_d[si * ST + j * 128:si * ST + (j + 1) * 128,
                                      h * W:(h + 1) * W],
                            in_=outt[:],
                        )
                return run

            for h in range(HL):
                ktile = kt_a if h < 2 else kt_b
                qtile = qt_h[h]
                for si in range(NS):
                    ssl = slice(si * ST, (si + 1) * ST)
                    ctx_ps = cxp.tile([65, ST], F32, name="ctx_ps", tag="ctx_ps")
                    prev = None          # (ex_tile, first_t)
                    n_acc = 0

                    def ctx_mms(ex, t0, last):
                        nonlocal n_acc
                        for j in range(exw):
                            nc.tensor.matmul(
                                ctx_ps[:], vaug[t0 + j][:, h, :],
                                ex[:, j * ST:(j + 1) * ST],
                                start=(n_acc == 0),
                                stop=(last and j == exw - 1),
                            )
                            n_acc += 1

                    for tp in range(NT // exw):
                        t0 = tp * exw
                        sc_ps = scp.tile([128, ST * exw], F32, name="sc_ps",
                                         tag="sc_ps")
                        for j in range(exw):
                            nc.tensor.matmul(
                                sc_ps[:, j * ST:(j + 1) * ST],
                                ktile[:, (t0 + j) * 128:(t0 + j + 1) * 128],
                                qtile[:, ssl],
                                start=True, stop=True,
                            )
                        ex = exp_pool.tile([128, ST * exw], F32R, name="ex",
                                           tag="ex")
                        nc.scalar.activation(
                            ex[:], sc_ps[:], AF.Exp,
                            bias=(mb[:, t0:t0 + 1] if has_mask else 0.0),
                            scale=0.125,
                        )
                        if prev is not None:
                            ctx_mms(prev[0], prev[1], last=False)
                        prev = (ex, t0)
                        if tp == 1:
                            flush_epilogue()
                    ctx_mms(prev[0], prev[1], last=True)
                    pending.append(epilogue(h, si, ctx_ps))
            flush_epilogue()


def _build(has_bias, has_mask):
    nc = bacc.Bacc(
        "TRN2", target_bir_lowering=False, debug=False, num_devices=N_CORES
    )
    xt_d = nc.dram_tensor("xt", [D, S], F32R, kind="ExternalInput").ap()
    wq_d = nc.dram_tensor("wq", [D + 1, DH], F32R, kind="ExternalInput").ap()
    wk_d = nc.dram_tensor("wk", [D + 1, DH], F32R, kind="ExternalInput").ap()
    wv_d = nc.dram_tensor("wv", [D + 1, VPAD], F32R, kind="ExternalInput").ap()
    id_d = nc.dram_tensor("ident", [128, 128], F32, kind="ExternalInput").ap()
    on_d = nc.dram_tensor("onesd", [128, ST], F32R, kind="ExternalInput").ap()
    zr_d = nc.dram_tensor("zerod", [64, S], F32R, kind="ExternalInput").ap()
    mb_d = (
        nc.dram_tensor("mb", [128, NT], F32, kind="ExternalInput").ap()
        if has_mask else None
    )
    out_d = nc.dram_tensor("out", [S, DH], F32, kind="ExternalOutput").ap()

    with tile.TileContext(nc) as tc:
        _emit(tc, (xt_d, wq_d, wk_d, wv_d, id_d, on_d, zr_d, mb_d, out_d),
              has_bias, has_mask)
    nc.compile()
    return nc


_NC_CACHE = {}


def _get_nc(has_bias, has_mask):
    key = (has_bias, has_mask)
    if key not in _NC_CACHE:
        _NC_CACHE[key] = _build(has_bias, has_mask)
    return _NC_CACHE[key]


def _in_maps(x, Wq, bq, Wk, bk, Wv, bv, mask, has_bias, has_mask):
    ident = np.eye(128, dtype=np.float32)
    xt_by_b = [np.ascontiguousarray(x[b].T) for b in range(B)]
    mb_by_b = [
        np.ascontiguousarray(
            ((np.asarray(mask[b]) == 0).astype(np.float32) * np.float32(-1e30))
            .reshape(NT, 128).T
        )
        for b in range(B)
    ]
    maps = []
    for c in range(N_CORES):
        b, g = divmod(c, N_CORES // B)
        lo = g * DH
        wq_a = np.empty((D + 1, DH), np.float32)
        wq_a[:D] = Wq[lo:lo + DH, :].T
        wq_a[D] = bq[lo:lo + DH]
        wk_a = np.empty((D + 1, DH), np.float32)
        wk_a[:D] = Wk[lo:lo + DH, :].T
        wk_a[D] = bk[lo:lo + DH]
        wv_a = np.zeros((D + 1, VPAD), np.float32)
        wv_a[:D, :DH] = Wv[lo:lo + DH, :].T
        wv_a[D, :DH] = bv[lo:lo + DH]
        m = {
            "xt": _round_f32r(xt_by_b[b]), "wq": _round_f32r(wq_a),
            "wk": _round_f32r(wk_a), "wv": _round_f32r(wv_a), "ident": ident,
            "onesd": np.ones((128, ST), np.float32),
            "zerod": np.zeros((64, S), np.float32),
        }
        if has_mask:
            m["mb"] = mb_by_b[b]
        maps.append(m)
    return maps


def _install_ntff_hook():
    """Best-effort: make trace=True work under axon by supplying the
    antenv.axon_hooks shim the boot code degrades without."""
    import types

    try:
        from antenv.axon_hooks import get_axon_ntff_profile_hook  # noqa: F401
        return True
    except ImportError:
        pass
    try:
        import antenv
        from trn_agent_boot.trn_boot import _ntff_profile_via_ctypes

        hook = _ntff_profile_via_ctypes("/opt/axon/libaxon_pjrt.so")
        if hook is None:
            return False
        mod = types.ModuleType("antenv.axon_hooks")
        state = {"hook": hook}
        mod.get_axon_ntff_profile_hook = lambda: state["hook"]
        mod.set_axon_ntff_profile_hook = lambda h: state.update(hook=h)
        sys.modules["antenv.axon_hooks"] = mod
        antenv.axon_hooks = mod
        return True
    except Exception:
        return False


def _run(x, Wq, bq, Wk, bk, Wv, bv, mask, trace=False):
    if trace:
        trace = _install_ntff_hook()
    x = np.ascontiguousarray(np.asarray(x, np.float32))
    Wq = np.asarray(Wq, np.float32)
    Wk = np.asarray(Wk, np.float32)
    Wv = np.asarray(Wv, np.float32)
    bq = np.asarray(bq, np.float32)
    bk = np.asarray(bk, np.float32)
    bv = np.asarray(bv, np.float32)
    has_bias = bool(np.any(bq) or np.any(bk) or np.any(bv))
    has_mask = bool((np.asarray(mask) == 0).any())
    nc = _get_nc(has_bias, has_mask)
    maps = _in_maps(x, Wq, bq, Wk, bk, Wv, bv, mask, has_bias, has_mask)
    res = run_bass_kernel_spmd(nc, maps, list(range(N_CORES)), trace=trace)
    out = np.empty((B, S, D), np.float32)
    for c in range(N_CORES):
        b, g = divmod(c, N_CORES // B)
        out[b, :, g * DH:(g + 1) * DH] = res.results[c]["out"]
    return out, res


def kernel(x, Wq, bq, Wk, bk, Wv, bv, mask):
    out, _ = _run(x, Wq, bq, Wk, bk, Wv, bv, mask)
    return out


# revision 10
# speedup vs baseline: 1.7766x; 1.0806x over previous
"""BERT self-attention (B=2, S=2048, D=768, H=12) on 8 trn2 NeuronCores.

Sharding: core c -> batch b = c//4, head group g = c%4 (3 heads each).
Attention is fully local per core; no collectives.

Per-core program (all f32 storage; matmuls run in float32r fast mode):
  qT/kT[dout, s] = W^T.T @ x^T          (projection, transposed orientation)
  v[t, w]        = x^T.T @ Wv^T         (natural orientation, +ones column)
  scoresT[t, s]  = kT_h.T @ qT_h        (per head, per 128-t-block)
  expS           = exp(scoresT/8 + maskbias[t])   (one ACT pass, PSUM->SBUF)
  ctxT_aug[65,s] = v_aug.T @ expS accumulated over t   (row 64 = sum of exp)
  ctx[s, 65]     = PE-transpose(ctxT_aug); out = ctx[:, :64] / ctx[:, 64]
"""

import sys

import numpy as np

_TRN_REPO = "/opt/trn_rl_repo"
if _TRN_REPO not in sys.path:
    sys.path.insert(0, _TRN_REPO)

import concourse.tile as tile  # noqa: E402
from concourse import bacc, mybir  # noqa: E402
from concourse.bass_utils import run_bass_kernel_spmd  # noqa: E402

F32 = mybir.dt.float32
F32R = mybir.dt.float32r
AF = mybir.ActivationFunctionType

B, S, D = 2, 2048, 768
H_TOT, W = 12, 64
N_CORES = 8
HL = 3                # heads per core
DH = HL * W           # 192 local output dims
KC = D // 128         # 6 contraction chunks of 128
ST = 512              # s-tile (matmul moving free dim)
NS = S // ST          # 4 s-tiles
NT = S // 128         # 16 t-blocks
VPAD = 256            # v-projection free dim padded so float32r runs 1 cyc/row


def _round_f32r(a):
    """Round-to-nearest-even fp32 -> fp32r (11-bit mantissa, value kept in
    the top 20 bits of the word) so DMA'd data is already fp32r-valid."""
    u = np.ascontiguousarray(a, np.float32).view(np.uint32).copy()
    u += np.uint32(0x7FF) + ((u >> np.uint32(12)) & np.uint32(1))
    u &= np.uint32(0xFFFFF000)
    return u.view(np.float32)


def _emit(tc, aps, has_bias, has_mask):
    nc = tc.nc
    xt_d, wq_d, wk_d, wv_d, id_d, on_d, zr_d, mb_d, out_d = aps

    from contextlib import ExitStack

    with ExitStack() as ctx:
        const = ctx.enter_context(tc.tile_pool(name="const", bufs=1))

        ident = const.tile([128, 128], F32, name="ident", tag="ident")
        nc.sync.dma_start(out=ident[:], in_=id_d[:, :])

        ones = const.tile([1, ST], F32R, name="ones", tag="ones")
        nc.sync.dma_start(out=ones[:], in_=on_d[0:1, :])

        mb = None
        if has_mask:
            mb = const.tile([128, NT], F32, name="mb", tag="mb")
            nc.sync.dma_start(out=mb[:], in_=mb_d[:, :])

        # x^T tiles: 6 chunks of [128 d, 2048 s], loaded per s-column-block so
        # compute can start before the whole 6.3MB lands.
        xt = []
        for c in range(KC):
            t = const.tile([128, S], F32R, name=f"xt{c}", tag=f"xt{c}")
            xt.append(t)
        dmae = [nc.sync, nc.scalar, nc.gpsimd]

        def load_xt_cols(si):
            for c in range(KC):
                dmae[c % 3].dma_start(
                    out=xt[c][:, si * ST:(si + 1) * ST],
                    in_=xt_d[c * 128:(c + 1) * 128, si * ST:(si + 1) * ST],
                )

        def w_tiles(name, ncols):
            chunks = []
            for c in range(KC):
                t = const.tile([128, ncols], F32R, name=f"{name}{c}", tag=f"{name}{c}")
                chunks.append(t)
            brow = const.tile([1, ncols], F32R, name=f"{name}b", tag=f"{name}b")
            return chunks, brow

        wq, wqb = w_tiles("wq", DH)
        wk, wkb = w_tiles("wk", DH)
        wv, wvb = w_tiles("wv", VPAD)
        # interleave so the c=0 projection matmuls can start after ~700KB
        for c in range(KC):
            dmae[c % 3].dma_start(
                out=xt[c][:, 0:ST], in_=xt_d[c * 128:(c + 1) * 128, 0:ST])
            for w_c, w_d in ((wq, wq_d), (wk, wk_d), (wv, wv_d)):
                dmae[(c + 1) % 3].dma_start(
                    out=w_c[c][:], in_=w_d[c * 128:(c + 1) * 128, :])
        if has_bias:
            for brow, w_d, ncols in ((wqb, wq_d, DH), (wkb, wk_d, DH),
                                     (wvb, wv_d, VPAD)):
                nc.sync.dma_start(out=brow[:], in_=w_d[D:D + 1, :])
        for si in range(1, NS):
            load_xt_cols(si)

        # Projection outputs (persistent). q tiles are zero-padded on the
        # complementary 64 partitions so every scores matmul runs K=128
        # (no PE row-mode switches mid-loop).
        qt_h = []
        for h in range(HL):
            t = const.tile([128, S], F32R, name=f"qt_h{h}", tag=f"qt_h{h}")
            qt_h.append(t)
        kt_a = const.tile([128, S], F32R, name="kt_a", tag="kt_a")
        kt_b = const.tile([128, S], F32R, name="kt_b", tag="kt_b")
        nc.sync.dma_start(out=qt_h[0][64:128, :], in_=zr_d[:, :])
        nc.scalar.dma_start(out=qt_h[1][0:64, :], in_=zr_d[:, :])
        nc.gpsimd.dma_start(out=qt_h[2][64:128, :], in_=zr_d[:, :])
        nc.gpsimd.dma_start(out=kt_b[64:128, :], in_=zr_d[:, :])
        vaug = []
        for t in range(NT):
            va = const.tile([128, HL, W + 1], F32R, name=f"vaug{t}", tag=f"vaug{t}")
            nc.sync.dma_start(
                out=va[:, :, W:W + 1],
                in_=on_d[0:128, 0:HL].rearrange("p (a b) -> p a b", b=1),
            )
            vaug.append(va)

        # ---- Phase A: projections -------------------------------------
        with tc.tile_pool(name="qkpsum", bufs=2, space="PSUM") as qkp, \
             tc.tile_pool(name="vpsum", bufs=2, space="PSUM") as vps:
            for si in range(NS):
                ssl = slice(si * ST, (si + 1) * ST)
                for chunks, brow, which in ((wq, wqb, "q"), (wk, wkb, "k")):
                    psA = qkp.tile([128, ST], F32, name="psA", tag="psA")
                    psB = qkp.tile([64, ST], F32, name="psB", tag="psB")
                    for c in range(KC):
                        nc.tensor.matmul(
                            psA[:], chunks[c][:, 0:128], xt[c][:, ssl],
                            start=(c == 0), stop=(c == KC - 1 and not has_bias),
                        )
                    if has_bias:
                        nc.tensor.matmul(
                            psA[:], brow[:, 0:128], ones[:],
                            start=False, stop=True,
                        )
                    for c in range(KC):
                        nc.tensor.matmul(
                            psB[:], chunks[c][:, 128:DH], xt[c][:, ssl],
                            start=(c == 0), stop=(c == KC - 1 and not has_bias),
                        )
                    if has_bias:
                        nc.tensor.matmul(
                            psB[:], brow[:, 128:DH], ones[:],
                            start=False, stop=True,
                        )
                    if which == "q":
                        nc.scalar.copy(qt_h[0][0:64, ssl], psA[0:64, :])
                        nc.scalar.copy(qt_h[1][64:128, ssl], psA[64:128, :])
                        nc.scalar.copy(qt_h[2][0:64, ssl], psB[:, :])
                    else:
                        nc.scalar.copy(kt_a[:, ssl], psA[:])
                        nc.scalar.copy(kt_b[0:64, ssl], psB[:, :])

                # v for the 4 t-blocks inside this s range
                for t in range(si * NS, (si + 1) * NS):
                    tsl = slice(t * 128, (t + 1) * 128)
                    psV = vps.tile([128, VPAD], F32, name="psV", tag="psV")
                    for c in range(KC):
                        nc.tensor.matmul(
                            psV[:], xt[c][:, tsl], wv[c][:],
                            start=(c == 0), stop=(c == KC - 1 and not has_bias),
                        )
                    if has_bias:
                        nc.tensor.matmul(
                            psV[:], ones[:, 0:128], wvb[:],
                            start=False, stop=True,
                        )
                    nc.vector.tensor_copy(
                        vaug[t][:, :, 0:W],
                        psV[:, 0:DH].rearrange("p (h w) -> p h w", h=HL),
                    )

        # ---- Phase B: attention ---------------------------------------
        exw = 1 if has_mask else 2   # t-blocks per exp tile
        with tc.tile_pool(name="scps", bufs=3, space="PSUM") as scp, \
             tc.tile_pool(name="ctxps", bufs=1, space="PSUM") as cxp, \
             tc.tile_pool(name="trps", bufs=1, space="PSUM") as trp, \
             tc.tile_pool(name="expool", bufs=3) as exp_pool, \
             tc.tile_pool(name="ctxsb", bufs=2) as csb, \
             tc.tile_pool(name="outsb", bufs=3) as osb:

            pending = []  # deferred epilogue from the previous (h, si)

            def flush_epilogue():
                while pending:
                    pending.pop(0)()

            def epilogue(h, si, ctx_ps):
                def run():
                    ctx_sb = csb.tile([65, ST], F32, name="ctx_sb", tag="ctx_sb")
                    nc.vector.tensor_copy(ctx_sb[:], ctx_ps[:])
                    for j in range(NS):
                        jsl = slice(j * 128, (j + 1) * 128)
                        tr_ps = trp.tile([128, 65], F32, name="tr_ps", tag="tr_ps")
                        nc.tensor.transpose(
                            tr_ps[:], ctx_sb[:, jsl], ident[0:65, 0:65]
                        )
                        rec = osb.tile([128, 1], F32, name="rec", tag="rec")
                        nc.vector.reciprocal(rec[:], tr_ps[:, W:W + 1])
                        outt = osb.tile([128, W], F32, name="outt", tag="outt")
                        nc.vector.tensor_scalar_mul(outt[:], tr_ps[:, 0:W], rec[:])
                        nc.sync.dma_start(
                            out=out_d[si * ST + j * 128:si * ST + (j + 1) * 128,
                                      h * W:(h + 1) * W],
                            in_=outt[:],
                        )
                return run

            for h in range(HL):
                ktile = kt_a if h < 2 else kt_b
                qtile = qt_h[h]
                for si in range(NS):
                    ssl = slice(si * ST, (si + 1) * ST)
                    ctx_ps = cxp.tile([65, ST], F32, name="ctx_ps", tag="ctx_ps")
                    prev = None          # (ex_tile, first_t)
                    n_acc = 0

                    def ctx_mms(ex, t0, last):
                        nonlocal n_acc
                        for j in range(exw):
                            nc.tensor.matmul(
                                ctx_ps[:], vaug[t0 + j][:, h, :],
                                ex[:, j * ST:(j + 1) * ST],
                                start=(n_acc == 0),
                                stop=(last and j == exw - 1),
                            )
                            n_acc += 1

                    for tp in range(NT // exw):
                        t0 = tp * exw
                        sc_ps = scp.tile([128, ST * exw], F32, name="sc_ps",
                                         tag="sc_ps")
                        for j in range(exw):
                            nc.tensor.matmul(
                                sc_ps[:, j * ST:(j + 1) * ST],
                                ktile[:, (t0 + j) * 128:(t0 + j + 1) * 128],
                                qtile[:, ssl],
                                start=True, stop=True,
                            )
                        ex = exp_pool.tile([128, ST * exw], F32R, name="ex",
                                           tag="ex")
                        nc.scalar.activation(
                            ex[:], sc_ps[:], AF.Exp,
                            bias=(mb[:, t0:t0 + 1] if has_mask else 0.0),
                            scale=0.125,
                        )
                        if prev is not None:
                            ctx_mms(prev[0], prev[1], last=False)
                        prev = (ex, t0)
                        if tp == 1:
                            flush_epilogue()
                    ctx_mms(prev[0], prev[1], last=True)
                    pending.append(epilogue(h, si, ctx_ps))
            flush_epilogue()


def _build(has_bias, has_mask):
    nc = bacc.Bacc(
        "TRN2", target_bir_lowering=False, debug=False, num_devices=N_CORES
    )
    xt_d = nc.dram_tensor("xt", [D, S], F32R, kind="ExternalInput").ap()
    wq_d = nc.dram_tensor("wq", [D + 1, DH], F32R, kind="ExternalInput").ap()
    wk_d = nc.dram_tensor("wk", [D + 1, DH], F32R, kind="ExternalInput").ap()
    wv_d = nc.dram_tensor("wv", [D + 1, VPAD], F32R, kind="ExternalInput").ap()
    id_d = nc.dram_tensor("ident", [128, 128], F32, kind="ExternalInput").ap()
    on_d = nc.dram_tensor("onesd", [128, ST], F32R, kind="ExternalInput").ap()
    zr_d = nc.dram_tensor("zerod", [64, S], F32R, kind="ExternalInput").ap()
    mb_d = (
        nc.dram_tensor("mb", [128, NT], F32, kind="ExternalInput").ap()
        if has_mask else None
    )
    out_d = nc.dram_tensor("out", [S, DH], F32, kind="ExternalOutput").ap()

    with tile.TileContext(nc) as tc:
        _emit(tc, (xt_d, wq_d, wk_d, wv_d, id_d, on_d, zr_d, mb_d, out_d),
              has_bias, has_mask)
    nc.compile()
    return nc


_NC_CACHE = {}


def _get_nc(has_bias, has_mask):
    key = (has_bias, has_mask)
    if key not in _NC_CACHE:
        _NC_CACHE[key] = _build(has_bias, has_mask)
    return _NC_CACHE[key]


def _in_maps(x, Wq, bq, Wk, bk, Wv, bv, mask, has_bias, has_mask):
    ident = np.eye(128, dtype=np.float32)
    xt_by_b = [np.ascontiguousarray(x[b].T) for b in range(B)]
    mb_by_b = [
        np.ascontiguousarray(
            ((np.asarray(mask[b]) == 0).astype(np.float32) * np.float32(-1e30))
            .reshape(NT, 128).T
        )
        for b in range(B)
    ]
    maps = []
    for c in range(N_CORES):
        b, g = divmod(c, N_CORES // B)
        lo = g * DH
        wq_a = np.empty((D + 1, DH), np.float32)
        wq_a[:D] = Wq[lo:lo + DH, :].T
        wq_a[D] = bq[lo:lo + DH]
        wk_a = np.empty((D + 1, DH), np.float32)
        wk_a[:D] = Wk[lo:lo + DH, :].T
        wk_a[D] = bk[lo:lo + DH]
        wv_a = np.zeros((D + 1, VPAD), np.float32)
        wv_a[:D, :DH] = Wv[lo:lo + DH, :].T
        wv_a[D, :DH] = bv[lo:lo + DH]
        m = {
            "xt": _round_f32r(xt_by_b[b]), "wq": _round_f32r(wq_a),
            "wk": _round_f32r(wk_a), "wv": _round_f32r(wv_a), "ident": ident,
            "onesd": np.ones((128, ST), np.float32),
            "zerod": np.zeros((64, S), np.float32),
        }
        if has_mask:
            m["mb"] = mb_by_b[b]
        maps.append(m)
    return maps


def _install_ntff_hook():
    """Best-effort: make trace=True work under axon by supplying the
    antenv.axon_hooks shim the boot code degrades without."""
    import types

    try:
        from antenv.axon_hooks import get_axon_ntff_profile_hook  # noqa: F401
        return True
    except ImportError:
        pass
    try:
        import antenv
        from trn_agent_boot.trn_boot import _ntff_profile_via_ctypes

        hook = _ntff_profile_via_ctypes("/opt/axon/libaxon_pjrt.so")
        if hook is None:
            return False
        mod = types.ModuleType("antenv.axon_hooks")
        state = {"hook": hook}
        mod.get_axon_ntff_profile_hook = lambda: state["hook"]
        mod.set_axon_ntff_profile_hook = lambda h: state.update(hook=h)
        sys.modules["antenv.axon_hooks"] = mod
        antenv.axon_hooks = mod
        return True
    except Exception:
        return False


def _run(x, Wq, bq, Wk, bk, Wv, bv, mask, trace=False):
    if trace:
        trace = _install_ntff_hook()
    x = np.ascontiguousarray(np.asarray(x, np.float32))
    Wq = np.asarray(Wq, np.float32)
    Wk = np.asarray(Wk, np.float32)
    Wv = np.asarray(Wv, np.float32)
    bq = np.asarray(bq, np.float32)
    bk = np.asarray(bk, np.float32)
    bv = np.asarray(bv, np.float32)
    has_bias = bool(np.any(bq) or np.any(bk) or np.any(bv))
    has_mask = bool((np.asarray(mask) == 0).any())
    nc = _get_nc(has_bias, has_mask)
    maps = _in_maps(x, Wq, bq, Wk, bk, Wv, bv, mask, has_bias, has_mask)
    res = run_bass_kernel_spmd(nc, maps, list(range(N_CORES)), trace=trace)
    out = np.empty((B, S, D), np.float32)
    for c in range(N_CORES):
        b, g = divmod(c, N_CORES // B)
        out[b, :, g * DH:(g + 1) * DH] = res.results[c]["out"]
    return out, res


def kernel(x, Wq, bq, Wk, bk, Wv, bv, mask):
    out, _ = _run(x, Wq, bq, Wk, bk, Wv, bv, mask)
    return out
